# revision 1
# baseline (speedup 1.0000x reference)
"""Trainium2 Bass kernel for nn_DeformableCrossAttention.

Sharding: data-parallel over batch B=8 across 8 NeuronCores (one sample per
core).  Inside each core:
  - offset MLP in fp32r (fp32 with 12-bit mantissa; full-rate 1 cyc/row on
    the PE vs 4 for fp32 -- position precision needs ~1e-4, fp32r gives it)
  - attention MLP + out-projection in bf16
  - v = context @ Wv single-pass bf16, quarter-split over heads so gathers
    for head group g start once that quarter of v is stored
  - bilinear sampling via per-head SWDGE dma_gather of 512B QUAD chunks
    (all 4 bilinear corners x 64 dh bf16) from 4x-shifted bf16 planes; the
    gather is descriptor-generation-bound (~7 ns/desc), so one descriptor
    per sample point is the key geometry
  - attention-weighted bilinear reduce on DVE (elementwise mult with
    factored corner weights + segmented add-reduce, bf16 out for 2x mode)
  - z transposed via PE (bf16), out-projection bf16, emitted transposed;
    host transposes back.

Self-contained: hardcodes all shapes from the problem spec.
"""
import sys
sys.path.insert(0, "/opt/trn_rl_repo")

import numpy as np
import concourse.bass as bass
import concourse.mybir as mybir
import concourse.tile as tile
from concourse import bacc
from concourse.bass_utils import run_bass_kernel_spmd
from concourse.masks import make_identity

F32 = mybir.dt.float32
F32R = mybir.dt.float32r
BF16 = mybir.dt.bfloat16
I16 = mybir.dt.int16
I32 = mybir.dt.int32
AF = mybir.ActivationFunctionType
ALU = mybir.AluOpType
AX = mybir.AxisListType

B, N, DIM = 8, 256, 1024
HEADS, DH, P = 16, 64, 8
HS = WS = 64
CTX = HS * WS            # 4096
INNER = HEADS * DH       # 1024
KT = DIM // 128          # 8 k-tiles
PLANE = CTX * DH         # per-head v plane elements (262144)

V_PASSES = int(__import__('os').environ.get('V_PASSES', '1'))  # 1: bf16, 3: bf16 hi/lo split (hh + hl + lh)
# timing bisection: run only pipeline stages up to this letter (d/e/f/g)
STAGE_CAP = __import__('os').environ.get('STAGE_CAP', 'g')
GONLY = __import__('os').environ.get('GONLY', '0') == '1'  # stage F: gather only
SPKT = __import__('os').environ.get('SPKT', '0') == '1'    # gather single_packet
SKIP_E = __import__('os').environ.get('SKIP_E', '0') == '1'  # timing: skip v matmul
GHEADS = int(__import__('os').environ.get('GHEADS', '16'))   # timing: gather count
GB512 = __import__('os').environ.get('GB512', '0') == '1'    # timing: 512B gather elems
CTX_SUP = 512            # ctx supertile rows
N_SUP = CTX // CTX_SUP   # 8 supertiles
M_PER_SUP = CTX_SUP // 128

_CACHE = {}


def _ap(t, offset, dims):
    return bass.AP(t.ap().tensor if hasattr(t, "ap") else t.tensor, offset, dims)


def _sap(tile_obj, extra, dims, part=None):
    """Sub-AP of an SBUF tile: keep its partition dim, custom free dims,
    extra offset in elements.  part=(off, cnt) slices the partition dim."""
    a = tile_obj[:] if not isinstance(tile_obj, bass.AP) else tile_obj
    if part is None:
        return bass.AP(a.tensor, a.offset + extra, [list(a.ap[0])] + dims)
    pstride = a.ap[0][0]
    return bass.AP(a.tensor, a.offset + extra + part[0] * pstride,
                   [[pstride, part[1]]] + dims)


def _build(repeat=1):
    nc = bacc.Bacc("TRN2", target_bir_lowering=False, debug=False)

    # ---------------- I/O ----------------
    ctxT_hi = nc.dram_tensor("ctxT_hi", [DIM, CTX], BF16, kind="ExternalInput")
    Wv_hi = nc.dram_tensor("Wv_hi", [DIM, INNER], BF16, kind="ExternalInput")
    if V_PASSES == 3:
        ctxT_lo = nc.dram_tensor("ctxT_lo", [DIM, CTX], BF16,
                                 kind="ExternalInput")
        Wv_lo = nc.dram_tensor("Wv_lo", [DIM, INNER], BF16,
                               kind="ExternalInput")
    else:
        ctxT_lo = Wv_lo = None
    # off path runs in fp32r (fp32 with 12-bit mantissa; full-rate on PE);
    # att + out-proj paths run in bf16.
    xoffT = nc.dram_tensor("xoffT", [DIM, N], F32R, kind="ExternalInput")
    xattT = nc.dram_tensor("xattT", [DIM, N], BF16, kind="ExternalInput")
    W_off1 = nc.dram_tensor("W_off1", [DIM, DIM], F32R, kind="ExternalInput")
    b_off1 = nc.dram_tensor("b_off1", [DIM], F32, kind="ExternalInput")
    W_off2p = nc.dram_tensor("W_off2p", [DIM, 256], F32R, kind="ExternalInput")
    b_off2p = nc.dram_tensor("b_off2p", [256], F32, kind="ExternalInput")
    W_att1 = nc.dram_tensor("W_att1", [DIM, DIM], BF16, kind="ExternalInput")
    b_att1 = nc.dram_tensor("b_att1", [DIM], F32, kind="ExternalInput")
    W_att2 = nc.dram_tensor("W_att2", [DIM, 128], BF16, kind="ExternalInput")
    b_att2 = nc.dram_tensor("b_att2", [128], F32, kind="ExternalInput")
    W_out = nc.dram_tensor("W_out", [INNER, DIM], BF16, kind="ExternalInput")
    b_out = nc.dram_tensor("b_out", [DIM], F32, kind="ExternalInput")

    outT = nc.dram_tensor("outT", [DIM, N], F32, kind="ExternalOutput")
    import os as _os
    DBG = _os.environ.get("KDBG", "0") == "1"
    if DBG:
        dbg_x = nc.dram_tensor("dbg_x", [128, 2048], F32, kind="ExternalOutput")
        dbg_h1 = nc.dram_tensor("dbg_h1", [128, N], F32, kind="ExternalOutput")
        dbg_lx = nc.dram_tensor("dbg_lx", [128, N], F32, kind="ExternalOutput")
        dbg_w = nc.dram_tensor("dbg_w", [128, 1024], F32, kind="ExternalOutput")

    # DRAM scratch.  v planes are bf16 QUAD chunks: chunk s = positions
    # {s, s+1, s+64, s+65} x 64 dh = 512B, so ONE SWDGE descriptor per
    # sample point fetches all 4 bilinear corners (the gather is
    # descriptor-generation-bound at ~7ns/desc, so halving descriptors
    # halves the gather wall time).
    PLANE4 = CTX * 256
    v_dram = nc.dram_tensor("v_dram", [HEADS, PLANE4], BF16)
    # idxC[q, h, p, m] int16 (p-major: partition (h,p) stride 16 -> 1 DMA)
    idxC = nc.dram_tensor("idxC", [16, HEADS, P, 16], I16)

    vwrite_insts = []
    vwrite_quarter = [[], [], [], []]
    cwrite_insts = []
    gather_insts = []
    idxload_insts = []

    with tile.TileContext(nc) as tc:
        import contextlib
        with contextlib.ExitStack() as ctx:
            persist = ctx.enter_context(tc.tile_pool(name="persist", bufs=1))
            ws = ctx.enter_context(tc.tile_pool(name="wstream", bufs=2))
            h1p = ctx.enter_context(tc.tile_pool(name="h1p", bufs=1))
            ctxp = ctx.enter_context(tc.tile_pool(name="ctxp", bufs=1))
            vsbp = ctx.enter_context(tc.tile_pool(name="vsbp", bufs=2))
            gp = ctx.enter_context(tc.tile_pool(name="gp", bufs=2))
            wtp = ctx.enter_context(tc.tile_pool(name="wtp", bufs=1))
            scr = ctx.enter_context(tc.tile_pool(name="scr", bufs=1))
            mps = ctx.enter_context(tc.tile_pool(name="mps", bufs=2, space="PSUM"))
            vps = ctx.enter_context(tc.tile_pool(name="vps", bufs=2, space="PSUM"))
            tps = ctx.enter_context(tc.tile_pool(name="tps", bufs=2, space="PSUM"))
            if repeat > 1:
                ctx.enter_context(tc.For_i(0, repeat, 1))

            # ---------- persistent loads ----------
            def load_tiles(dram, rows, cols, dt, tag, ncols=None):
                ncols = cols if ncols is None else ncols
                ts_ = []
                for k in range(rows // 128):
                    t = persist.tile([128, ncols], dt, tag=f"{tag}_{k}")
                    nc.sync.dma_start(t[:], dram[k * 128:(k + 1) * 128, :])
                    ts_.append(t)
                return ts_

            wv_hi = load_tiles(Wv_hi, DIM, INNER, BF16, "wvh")
            wv_lo = load_tiles(Wv_lo, DIM, INNER, BF16, "wvl") if V_PASSES == 3 else None
            def load_wide(dram, cols, dt, tag):
                big = persist.tile([128, KT * cols], dt, tag=tag, name=tag)
                nc.sync.dma_start(
                    bass.AP(big[:].tensor, big[:].offset,
                            [[KT * cols, 128], [cols, KT], [1, cols]]),
                    bass.AP(dram.ap().tensor, 0,
                            [[cols, 128], [128 * cols, KT], [1, cols]]))
                return [big[:, k * cols:(k + 1) * cols] for k in range(KT)]

            woff2 = load_wide(W_off2p, 256, F32R, "wo2big")
            watt2 = load_wide(W_att2, 128, BF16, "wa2big")

            def load_bias(dram, n_elem, tag):
                k = n_elem // 128
                t = persist.tile([128, k], F32, tag=tag)
                nc.sync.dma_start(t[:], _ap(dram, 0, [[1, 128], [128, k]]))
                return t

            bo1 = load_bias(b_off1, DIM, "bo1")
            bo2 = load_bias(b_off2p, 256, "bo2")
            ba1 = load_bias(b_att1, DIM, "ba1")
            ba2 = load_bias(b_att2, 128, "ba2")
            bo = load_bias(b_out, DIM, "bo")

            ident = persist.tile([128, 128], F32, tag="ident")
            make_identity(nc, ident[:])
            identb = persist.tile([128, 128], BF16, tag="identb")
            make_identity(nc, identb[:])

            # ---------- MLP helper: yT[m] = act(sum_k W[k,m]^T @ xT[k] + b[m]) ----------
            def mlp_layer(w_dram, x_tiles, bias_tile, mtiles, act, out_tag,
                          pool, dt, out_dt):
                # w_dram is [DIM, mtiles*128]; one coalesced DMA per m-tile
                wcols = mtiles * 128
                outs = []
                for m in range(mtiles):
                    ps = mps.tile([128, N], F32, tag="mlp_ps")
                    wt = ws.tile([128, 1024], dt, tag="wst",
                                 name=f"wst_{out_tag}_{m}")
                    src_ap = bass.AP(w_dram.ap().tensor, m * 128,
                                     [[wcols, 128], [128 * wcols, KT], [1, 128]])
                    nc.sync.dma_start(
                        bass.AP(wt[:].tensor, wt[:].offset,
                                [[1024, 128], [128, KT], [1, 128]]),
                        src_ap)
                    for k in range(KT):
                        nc.tensor.matmul(ps[:], wt[:, k * 128:(k + 1) * 128],
                                         x_tiles[k][:],
                                         start=(k == 0), stop=(k == KT - 1))
                    o = pool.tile([128, N], out_dt, tag=f"{out_tag}_{m}")
                    nc.scalar.activation(o[:], ps[:], act,
                                         bias=bias_tile[:, m:m + 1])
                    outs.append(o)
                return outs

            def mlp_layer2(w_tiles, x_tiles, bias_tile, mtiles, act, out_tag,
                           use_dve_bias=False):
                outs = []
                for m in range(mtiles):
                    ps = mps.tile([128, N], F32, tag="mlp_ps")
                    for k in range(KT):
                        nc.tensor.matmul(ps[:], w_tiles[k][:, m * 128:(m + 1) * 128],
                                         x_tiles[k][:],
                                         start=(k == 0), stop=(k == KT - 1))
                    o = scr.tile([128, N], F32, tag=f"{out_tag}_{m}")
                    if use_dve_bias:
                        nc.vector.tensor_scalar(o[:], ps[:], bias_tile[:, m:m + 1],
                                                None, op0=ALU.add)
                    else:
                        nc.scalar.activation(o[:], ps[:], act,
                                             bias=bias_tile[:, m:m + 1])
                    outs.append(o)
                return outs

            # ---------- stage A: offset MLP (fp32r) ----------
            def load_xT(dram, dt, name):
                big = h1p.tile([128, KT * N], dt, tag="xt_big", name=name)
                nc.sync.dma_start(
                    bass.AP(big[:].tensor, big[:].offset,
                            [[KT * N, 128], [N, KT], [1, N]]),
                    bass.AP(dram.ap().tensor, 0,
                            [[N, 128], [128 * N, KT], [1, N]]))
                return [big[:, k * N:(k + 1) * N] for k in range(KT)]

            xoff_t = load_xT(xoffT, F32R, "xoff_big")
            h1 = mlp_layer(W_off1, xoff_t, bo1, KT, AF.Gelu, "h1", h1p,
                           dt=F32R, out_dt=F32R)
            # off2: 2 m-tiles -> lxT (cy=0), lyT (cy=1), tanh applied
            loff = mlp_layer2(woff2, h1, bo2, 2, AF.Tanh, "loff")
            lxT, lyT = loff

            # ---------- stage B: attention MLP (bf16) ----------
            xatt_t = load_xT(xattT, BF16, "xatt_big")
            g1 = mlp_layer(W_att1, xatt_t, ba1, KT, AF.Gelu, "h1", h1p,
                           dt=BF16, out_dt=BF16)
            attT = mlp_layer2(watt2, g1, ba2, 1, AF.Copy, "attT",
                              use_dve_bias=True)[0]

            # ---------- stage C: PE transposes to [n, hp] ----------
            def transpose_128x256(src, tag):
                halves = []
                for i in range(2):
                    pt = tps.tile([128, 128], F32, tag="trps")
                    nc.tensor.transpose(pt[:], src[:, i * 128:(i + 1) * 128],
                                        ident[:])
                    o = scr.tile([128, 128], F32, tag=f"{tag}_{i}")
                    nc.vector.tensor_copy(o[:], pt[:])
                    halves.append(o)
                return halves

            lx_n = transpose_128x256(lxT, "lxn")   # [n-tile][128, 128hp]
            ly_n = transpose_128x256(lyT, "lyn")
            att_n = transpose_128x256(attT, "attn")

            # ---------- stage D1: positions in [hp, n] for gather indices ----------
            # g' = tanh*31.5 + 31.0 ; min 62.4999 ; xi = round(g'), xf float
            def pos_chain_T(lt, tag):
                # g = clamp(tanh*31.5 + 31.5); xf = floor(g) exactly,
                # independent of the hw/sim cast rounding mode:
                # xi = cast(g); xf = float(xi); m = (g - xf < 0); xf -= m
                gp_ = scr.tile([128, N], F32, tag="tmpA", name=f"{tag}_g")
                nc.vector.tensor_scalar(gp_[:], lt[:], 31.5, 31.5,
                                        op0=ALU.mult, op1=ALU.add)
                nc.vector.tensor_scalar(gp_[:], gp_[:], 62.9999, 0.0,
                                        op0=ALU.min, op1=ALU.max)
                xi = scr.tile([128, N], I32, tag="tmpB", name=f"{tag}_i")
                nc.vector.tensor_copy(xi[:], gp_[:])
                xf = scr.tile([128, N], F32, tag=f"{tag}_f")
                nc.vector.tensor_copy(xf[:], xi[:])
                wr = scr.tile([128, N], F32, tag="tmpC", name=f"{tag}_wr")
                nc.vector.tensor_tensor(wr[:], gp_[:], xf[:], op=ALU.subtract)
                mneg = scr.tile([128, N], F32, tag="tmpD", name=f"{tag}_m")
                nc.vector.tensor_scalar(mneg[:], wr[:], 0.0, None, op0=ALU.is_lt)
                nc.vector.tensor_tensor(xf[:], xf[:], mneg[:], op=ALU.subtract)
                return xf

            xfT = pos_chain_T(lxT, "pxT")
            yfT = pos_chain_T(lyT, "pyT")
            idx0f = scr.tile([128, N], F32, tag="tmpA", name="idx0f")
            nc.vector.scalar_tensor_tensor(idx0f[:], yfT[:], 64.0, xfT[:],
                                           op0=ALU.mult, op1=ALU.add)
            ii = scr.tile([128, N], I16, tag="tmpB", name="idxi")
            nc.vector.tensor_copy(ii[:], idx0f[:])
            # free transpose: Sg[hp, q*16+m] = ii[hp, m*16+q]
            sg = scr.tile([128, N], I16, tag="sg_0")
            nc.vector.tensor_copy(
                sg[:], _sap(ii, 0, [[1, 16], [16, 16]]))
            # write to idxC[q, h, p, m]: partition (h,p) has constant
            # stride 16, so one DMA covers all 128 partitions x (q, m)
            dst = bass.AP(idxC.ap().tensor, 0,
                          [[16, 128], [2048, 16], [1, 16]])
            w = nc.sync.dma_start(dst, sg[:])
            cwrite_insts.append(w)

            # ---------- stage D2: lerp weights + softmax in [n, hp] ----------
            # t[n, col = h*32 + cy*16 + p*2 + half]
            t_tiles = []
            for nh in range(2):
                def frac_chain(src_t, tag, otag):
                    g_ = scr.tile([128, 128], F32, tag="tmpA", name=f"{tag}_g")
                    nc.vector.tensor_scalar(g_[:], src_t[:], 31.5, 31.5,
                                            op0=ALU.mult, op1=ALU.add)
                    nc.vector.tensor_scalar(g_[:], g_[:], 62.9999, 0.0,
                                            op0=ALU.min, op1=ALU.max)
                    i_ = scr.tile([128, 128], I32, tag="tmpB", name=f"{tag}_i")
                    nc.vector.tensor_copy(i_[:], g_[:])
                    f_ = scr.tile([128, 128], F32, tag="tmpC", name=f"{tag}_f")
                    nc.vector.tensor_copy(f_[:], i_[:])
                    wr_ = scr.tile([128, 128], F32, tag="tmpD", name=f"{tag}_wr")
                    nc.vector.tensor_tensor(wr_[:], g_[:], f_[:], op=ALU.subtract)
                    mn_ = scr.tile([128, 128], F32, tag="tmpA", name=f"{tag}_m")
                    nc.vector.tensor_scalar(mn_[:], wr_[:], 0.0, None, op0=ALU.is_lt)
                    w_ = scr.tile([128, 128], F32, tag=otag, name=f"{tag}_w")
                    nc.vector.tensor_tensor(w_[:], wr_[:], mn_[:], op=ALU.add)
                    return w_
                wx = frac_chain(lx_n[nh], f"fx{nh}", "wx")
                wy = frac_chain(ly_n[nh], f"fy{nh}", "wy")

                # softmax over p (groups of 8 along free)
                an = att_n[nh]
                mx = scr.tile([128, 16], F32, tag="mx", name=f"mx_{nh}")
                nc.vector.tensor_reduce(
                    mx[:], _sap(an, 0, [[8, 16], [1, 8]]),
                    axis=AX.X, op=ALU.max)
                ex = scr.tile([128, 128], F32, tag="tmpB", name=f"ex_{nh}")
                nc.vector.tensor_tensor(
                    _sap(ex, 0, [[8, 16], [1, 8]]),
                    _sap(an, 0, [[8, 16], [1, 8]]),
                    _sap(mx, 0, [[1, 16], [0, 8]]),
                    op=ALU.subtract)
                nc.scalar.activation(ex[:], ex[:], AF.Exp)
                sm = scr.tile([128, 16], F32, tag="sm", name=f"sm_{nh}")
                nc.vector.tensor_reduce(
                    sm[:], _sap(ex, 0, [[8, 16], [1, 8]]),
                    axis=AX.X, op=ALU.add)
                rs = scr.tile([128, 16], F32, tag="rs", name=f"rs_{nh}")
                nc.vector.reciprocal(rs[:], sm[:])
                aw = scr.tile([128, 128], F32, tag="tmpC", name=f"aw_{nh}")
                nc.vector.tensor_tensor(
                    _sap(aw, 0, [[8, 16], [1, 8]]),
                    _sap(ex, 0, [[8, 16], [1, 8]]),
                    _sap(rs, 0, [[1, 16], [0, 8]]),
                    op=ALU.mult)

                # u0 = aw*(1-wx) = aw - aw*wx ; u1 = aw*wx
                u1 = scr.tile([128, 128], F32, tag="tmpD", name=f"u1_{nh}")
                nc.vector.tensor_tensor(u1[:], aw[:], wx[:], op=ALU.mult)
                u0 = scr.tile([128, 128], F32, tag="tmpA", name=f"u0_{nh}")
                nc.vector.tensor_tensor(u0[:], aw[:], u1[:], op=ALU.subtract)
                cw1 = wy
                cw0 = scr.tile([128, 128], F32, tag="tmpB", name=f"cw0_{nh}")
                nc.vector.tensor_scalar(cw0[:], wy[:], -1.0, 1.0,
                                        op0=ALU.mult, op1=ALU.add)

                # t col = h*32 + p*4 + cy*2 + half (p-major, matching the
                # idxC item order p*32 + cy*16 + m)
                tt = scr.tile([128, 512], BF16, tag=f"tt_{nh}")
                for cyv, cw in ((0, cw0), (1, cw1)):
                    for half, u in ((0, u0), (1, u1)):
                        nc.vector.tensor_tensor(
                            _sap(tt, cyv * 2 + half, [[32, 16], [4, 8]]),
                            _sap(u, 0, [[8, 16], [1, 8]]),
                            _sap(cw, 0, [[8, 16], [1, 8]]),
                            op=ALU.mult)
                t_tiles.append(tt)

            if STAGE_CAP >= 'e' and not SKIP_E:
                # ---------- stage E: v matmul + store (head-half outer so the
                # gather for heads 0-7 overlaps the matmul of heads 8-15).
                # ctx supertiles are loaded once and stay resident for both
                # halves. ----------
                def load_ctx(dram, c0, tag):
                    big = ctxp.tile([128, KT * CTX_SUP], BF16, tag=tag, name=tag)
                    nc.sync.dma_start(
                        bass.AP(big[:].tensor, big[:].offset,
                                [[KT * CTX_SUP, 128], [CTX_SUP, KT], [1, CTX_SUP]]),
                        bass.AP(dram.ap().tensor, c0,
                                [[CTX, 128], [128 * CTX, KT], [1, CTX_SUP]]))
                    return [big[:, k * CTX_SUP:(k + 1) * CTX_SUP]
                            for k in range(KT)]

                ctx_hi_t = [load_ctx(ctxT_hi, sup * CTX_SUP, f"chbig_{sup}")
                            for sup in range(N_SUP)]
                ctx_lo_t = ([load_ctx(ctxT_lo, sup * CTX_SUP, f"clbig_{sup}")
                             for sup in range(N_SUP)] if V_PASSES == 3 else None)

                # quarter-split over heads: gathers for head group g can
                # start once that quarter of v is written (~25% into stage E).
                # Pair chunks are assembled in SBUF (vsb2) so each tile's
                # store is ONE dma with a 1KB contiguous run per partition.
                for qg in range(4):
                    for sup in range(N_SUP):
                        c0 = sup * CTX_SUP
                        chi = ctx_hi_t[sup]
                        for mm in range(M_PER_SUP):
                            msl = slice(mm * 128, (mm + 1) * 128)
                            ps = vps.tile([128, 256], F32, tag="vps",
                                          name=f"vps_{qg}_{sup}_{mm}")
                            for k in range(KT):
                                nc.tensor.matmul(
                                    ps[:], chi[k][:, msl],
                                    wv_hi[k][:, qg * 256:(qg + 1) * 256],
                                    start=(k == 0), stop=(k == KT - 1))
                            vsb = vsbp.tile([128, 256], BF16, tag="vsb",
                                            name=f"vsb_{qg}_{sup}_{mm}")
                            nc.scalar.copy(vsb[:], ps[:])
                            # quad plane: position p lands in chunk p-d at slot d
                            # for d in {0 (self), 1 (x+1), 64 (y+1), 65 (x+1,y+1)}
                            p0 = c0 + mm * 128
                            for dlt, soff in ((0, 0), (1, DH), (64, 2 * DH),
                                              (65, 3 * DH)):
                                if p0 == 0:
                                    dst = bass.AP(
                                        v_dram.ap().tensor,
                                        qg * 4 * PLANE4 + soff,
                                        [[256, 128 - dlt], [PLANE4, 4], [1, DH]])
                                    w = nc.sync.dma_start(
                                        dst, vsb[dlt:128, :] if dlt else vsb[:])
                                else:
                                    dst = bass.AP(
                                        v_dram.ap().tensor,
                                        qg * 4 * PLANE4 + (p0 - dlt) * 256 + soff,
                                        [[256, 128], [PLANE4, 4], [1, DH]])
                                    w = nc.sync.dma_start(dst, vsb[:])
                                vwrite_quarter[qg].append(w)
                                vwrite_insts.append(w)

            if STAGE_CAP >= 'f':
                # ---------- stage F: per-head gather + reduce ----------
                # z reuses the xt_big slot (dead after stage B MLPs); bf16 so the
                # segmented reduce qualifies for the DVE 2x_1P perf mode
                z = h1p.tile([128, 2048], BF16, tag="xt_big", name="z")  # col = nh*1024 + h*64 + d
                # all-heads idx tile [128, 4096]: col = h*256 + cy*128 + p*16 + m,
                # partitions = 8 replicas x 16 q
                all_idx = persist.tile([128, 2048], I16, tag="all_idx")
                for gi_ in range(8):
                    src = bass.AP(idxC.ap().tensor, 0, [[2048, 16], [1, 2048]])
                    ld = nc.sync.dma_start(all_idx[gi_ * 16:(gi_ + 1) * 16, :], src)
                    idxload_insts.append(ld)

                for h in range(GHEADS):
                    g = gp.tile([128, 16, 256], BF16, tag="g")
                    vsrc = bass.AP(v_dram.ap().tensor, h * PLANE4,
                                   [[256, CTX - 65], [1, 256]])
                    gi = nc.gpsimd.dma_gather(g[:], vsrc,
                                              all_idx[:, h * 128:(h + 1) * 128],
                                              2048, 2048,
                                              256, elem_step=256,
                                              single_packet=SPKT)
                    gather_insts.append(gi)

                    for nh in range(2 if not GONLY else 0):
                        wt = wtp.tile([128, 2048], BF16, tag="wt")
                        # wt[d*32 + p*4 + slot] = G[n, slab 2p+nh, slot*64+d] * t
                        nc.vector.tensor_tensor(
                            _sap(wt, 0, [[32, 64], [4, 8], [1, 4]]),
                            _sap(g, nh * 256, [[1, 64], [512, 8], [64, 4]]),
                            _sap(t_tiles[nh], h * 32, [[0, 64], [4, 8], [1, 4]]),
                            op=ALU.mult)
                        with nc.allow_low_precision(
                                reason="bf16 z: 2x DVE reduce; rounded to bf16 "
                                       "for the out-proj matmul anyway"):
                            nc.vector.tensor_reduce(
                                _sap(z, nh * 1024 + h * 64, [[1, 64]]),
                                _sap(wt, 0, [[32, 64], [1, 32]]),
                                axis=AX.X, op=ALU.add)

            if STAGE_CAP >= 'g':
                # ---------- stage G: z -> zT, out projection (bf16) ----------
                # zT reuses the wo2big slot (dead after stage A MLPs)
                zbig = persist.tile([128, KT * N], BF16, tag="wo2big", name="zbig")
                zT = [zbig[:, k * N:(k + 1) * N] for k in range(KT)]
                for nh in range(2):
                    for c in range(8):
                        pt = tps.tile([128, 128], BF16, tag="trps")
                        nc.tensor.transpose(
                            pt[:], z[:, nh * 1024 + c * 128: nh * 1024 + (c + 1) * 128],
                            identb[:])
                        nc.vector.tensor_copy(zT[c][:, nh * 128:(nh + 1) * 128], pt[:])

                for m in range(KT):
                    ps = mps.tile([128, N], F32, tag="mlp_ps")
                    wt = ws.tile([128, 1024], BF16, tag="wst", name=f"wst_out_{m}")
                    nc.sync.dma_start(
                        bass.AP(wt[:].tensor, wt[:].offset,
                                [[1024, 128], [128, KT], [1, 128]]),
                        bass.AP(W_out.ap().tensor, m * 128,
                                [[DIM, 128], [128 * DIM, KT], [1, 128]]))
                    for k in range(KT):
                        nc.tensor.matmul(ps[:], wt[:, k * 128:(k + 1) * 128],
                                         zT[k][:],
                                         start=(k == 0), stop=(k == KT - 1))
                    osb = scr.tile([128, N], F32, tag="osb")
                    nc.vector.tensor_scalar(osb[:], ps[:], bo[:, m:m + 1], None,
                                            op0=ALU.add)
                    nc.sync.dma_start(outT[m * 128:(m + 1) * 128, :], osb[:])

            # ---------- explicit DRAM deps (gather after v/idx writes) ----------
            from concourse.tile_rust import add_dep_helper
            for hh, gi in enumerate(gather_insts):
                for w in vwrite_quarter[hh // 4]:
                    add_dep_helper(gi.ins, w.ins, reason="gather after v write")
            for ld in idxload_insts:
                for w in cwrite_insts:
                    add_dep_helper(ld.ins, w.ins, reason="idx load after idxC write")

    nc.compile()
    return nc


def _prep_inputs(inputs):
    x = np.ascontiguousarray(np.asarray(inputs["x"], dtype=np.float32))
    context = np.asarray(inputs["context"], dtype=np.float32)
    Wv = np.asarray(inputs["Wv"], dtype=np.float32)
    W_off2 = np.asarray(inputs["W_off2"], dtype=np.float32)
    b_off2 = np.asarray(inputs["b_off2"], dtype=np.float32)

    def split(a):
        import ml_dtypes
        hi = a.astype(ml_dtypes.bfloat16)
        lo = (a - hi.astype(np.float32)).astype(ml_dtypes.bfloat16)
        return hi, lo

    Wv_hi, Wv_lo = split(Wv)

    cy, h, p = np.meshgrid(np.arange(2), np.arange(HEADS), np.arange(P),
                           indexing="ij")
    oldcol = (h * 16 + p * 2 + cy).reshape(-1)
    W_off2p = np.ascontiguousarray(W_off2[:, oldcol])
    b_off2p = np.ascontiguousarray(b_off2[oldcol])

    import ml_dtypes
    bf = lambda a: np.ascontiguousarray(
        np.asarray(a, np.float32).astype(ml_dtypes.bfloat16))
    common = {
        "Wv_hi": Wv_hi,
        "W_off1": np.ascontiguousarray(np.asarray(inputs["W_off1"], np.float32)),
        "b_off1": np.asarray(inputs["b_off1"], np.float32),
        "W_off2p": W_off2p, "b_off2p": b_off2p,
        "W_att1": bf(inputs["W_att1"]),
        "b_att1": np.asarray(inputs["b_att1"], np.float32),
        "W_att2": bf(inputs["W_att2"]),
        "b_att2": np.asarray(inputs["b_att2"], np.float32),
        "W_out": bf(inputs["W_out"]),
        "b_out": np.asarray(inputs["b_out"], np.float32),
    }

    xflat = x.reshape(B * N, DIM)
    n_idx = np.arange(N)
    in_maps = []
    for b in range(B):
        perm = (n_idx // 32) * 256 + (n_idx % 32) * 8 + b
        xoffT = np.ascontiguousarray(xflat[perm].T)
        xattT = bf(x[b].T)
        ctxT = np.ascontiguousarray(context[b].T)
        c_hi, c_lo = split(ctxT)
        m = dict(common)
        m.update({"ctxT_hi": c_hi, "xoffT": xoffT, "xattT": xattT})
        if V_PASSES == 3:
            m.update({"ctxT_lo": c_lo, "Wv_lo": Wv_lo})
        in_maps.append(m)
    return in_maps


def kernel(**inputs):
    if "nc" not in _CACHE:
        _CACHE["nc"] = _build()
    nc = _CACHE["nc"]
    in_maps = _prep_inputs(inputs)
    res = run_bass_kernel_spmd(nc, in_maps, list(range(8)))
    _CACHE["last_results"] = res
    out = np.stack([res.results[i]["outT"].T for i in range(B)], axis=0)
    return np.ascontiguousarray(out.astype(np.float32))



# revision 17
# speedup vs baseline: 1.0390x; 1.0390x over previous
"""Trainium2 Bass kernel for nn_DeformableCrossAttention.

Sharding: data-parallel over batch B=8 across 8 NeuronCores (one sample per
core).  Inside each core:
  - offset MLP in fp32r (fp32 with 12-bit mantissa; full-rate 1 cyc/row on
    the PE vs 4 for fp32 -- position precision needs ~1e-4, fp32r gives it)
  - attention MLP + out-projection in bf16
  - v = context @ Wv single-pass bf16, quarter-split over heads so gathers
    for head group g start once that quarter of v is stored
  - bilinear sampling via per-head SWDGE dma_gather of 512B QUAD chunks
    (all 4 bilinear corners x 64 dh bf16) from 4x-shifted bf16 planes; the
    gather is descriptor-generation-bound (~7 ns/desc), so one descriptor
    per sample point is the key geometry
  - attention-weighted bilinear reduce on DVE (elementwise mult with
    factored corner weights + segmented add-reduce, bf16 out for 2x mode)
  - z transposed via PE (bf16), out-projection bf16, emitted transposed;
    host transposes back.

Self-contained: hardcodes all shapes from the problem spec.
"""
import sys
sys.path.insert(0, "/opt/trn_rl_repo")

import numpy as np
import concourse.bass as bass
import concourse.mybir as mybir
import concourse.tile as tile
from concourse import bacc
from concourse.bass_utils import run_bass_kernel_spmd
from concourse.masks import make_identity

F32 = mybir.dt.float32
F32R = mybir.dt.float32r
BF16 = mybir.dt.bfloat16
I16 = mybir.dt.int16
I32 = mybir.dt.int32
AF = mybir.ActivationFunctionType
ALU = mybir.AluOpType
AX = mybir.AxisListType

B, N, DIM = 8, 256, 1024
HEADS, DH, P = 16, 64, 8
HS = WS = 64
CTX = HS * WS            # 4096
INNER = HEADS * DH       # 1024
KT = DIM // 128          # 8 k-tiles
PLANE = CTX * DH         # per-head v plane elements (262144)

V_PASSES = int(__import__('os').environ.get('V_PASSES', '1'))  # 1: bf16, 3: bf16 hi/lo split (hh + hl + lh)
# timing bisection: run only pipeline stages up to this letter (d/e/f/g)
STAGE_CAP = __import__('os').environ.get('STAGE_CAP', 'g')
GONLY = __import__('os').environ.get('GONLY', '0') == '1'  # stage F: gather only
SPKT = __import__('os').environ.get('SPKT', '0') == '1'    # gather single_packet
SKIP_E = __import__('os').environ.get('SKIP_E', '0') == '1'  # timing: skip v matmul
GHEADS = int(__import__('os').environ.get('GHEADS', '16'))   # timing: gather count
GB512 = __import__('os').environ.get('GB512', '0') == '1'    # timing: 512B gather elems
CTX_SUP = 512            # ctx supertile rows
N_SUP = CTX // CTX_SUP   # 8 supertiles
M_PER_SUP = CTX_SUP // 128

_CACHE = {}


def _ap(t, offset, dims):
    return bass.AP(t.ap().tensor if hasattr(t, "ap") else t.tensor, offset, dims)


def _sap(tile_obj, extra, dims, part=None):
    """Sub-AP of an SBUF tile: keep its partition dim, custom free dims,
    extra offset in elements.  part=(off, cnt) slices the partition dim."""
    a = tile_obj[:] if not isinstance(tile_obj, bass.AP) else tile_obj
    if part is None:
        return bass.AP(a.tensor, a.offset + extra, [list(a.ap[0])] + dims)
    pstride = a.ap[0][0]
    return bass.AP(a.tensor, a.offset + extra + part[0] * pstride,
                   [[pstride, part[1]]] + dims)


def _build(repeat=1):
    nc = bacc.Bacc("TRN2", target_bir_lowering=False, debug=False)

    # ---------------- I/O ----------------
    ctxT_hi = nc.dram_tensor("ctxT_hi", [DIM, CTX], BF16, kind="ExternalInput")
    Wv_hi = nc.dram_tensor("Wv_hi", [DIM, INNER], BF16, kind="ExternalInput")
    if V_PASSES == 3:
        ctxT_lo = nc.dram_tensor("ctxT_lo", [DIM, CTX], BF16,
                                 kind="ExternalInput")
        Wv_lo = nc.dram_tensor("Wv_lo", [DIM, INNER], BF16,
                               kind="ExternalInput")
    else:
        ctxT_lo = Wv_lo = None
    # off path runs in fp32r (fp32 with 12-bit mantissa; full-rate on PE);
    # att + out-proj paths run in bf16.
    xoffT = nc.dram_tensor("xoffT", [DIM, N], F32R, kind="ExternalInput")
    xattT = nc.dram_tensor("xattT", [DIM, N], BF16, kind="ExternalInput")
    W_off1 = nc.dram_tensor("W_off1", [DIM, DIM], F32R, kind="ExternalInput")
    b_off1 = nc.dram_tensor("b_off1", [DIM], F32, kind="ExternalInput")
    W_off2p = nc.dram_tensor("W_off2p", [DIM, 256], F32R, kind="ExternalInput")
    b_off2p = nc.dram_tensor("b_off2p", [256], F32, kind="ExternalInput")
    W_att1 = nc.dram_tensor("W_att1", [DIM, DIM], BF16, kind="ExternalInput")
    b_att1 = nc.dram_tensor("b_att1", [DIM], F32, kind="ExternalInput")
    W_att2 = nc.dram_tensor("W_att2", [DIM, 128], BF16, kind="ExternalInput")
    b_att2 = nc.dram_tensor("b_att2", [128], F32, kind="ExternalInput")
    W_out = nc.dram_tensor("W_out", [INNER, DIM], BF16, kind="ExternalInput")
    b_out = nc.dram_tensor("b_out", [DIM], F32, kind="ExternalInput")

    outT = nc.dram_tensor("outT", [DIM, N], F32, kind="ExternalOutput")
    import os as _os
    DBG = _os.environ.get("KDBG", "0") == "1"
    if DBG:
        dbg_x = nc.dram_tensor("dbg_x", [128, 2048], F32, kind="ExternalOutput")
        dbg_h1 = nc.dram_tensor("dbg_h1", [128, N], F32, kind="ExternalOutput")
        dbg_lx = nc.dram_tensor("dbg_lx", [128, N], F32, kind="ExternalOutput")
        dbg_w = nc.dram_tensor("dbg_w", [128, 1024], F32, kind="ExternalOutput")

    # DRAM scratch.  v planes are bf16 QUAD chunks: chunk s = positions
    # {s, s+1, s+64, s+65} x 64 dh = 512B, so ONE SWDGE descriptor per
    # sample point fetches all 4 bilinear corners (the gather is
    # descriptor-generation-bound at ~7ns/desc, so halving descriptors
    # halves the gather wall time).
    PLANE4 = CTX * 256
    GUARD = 65 * 256         # negative-chunk guard band (written, never read)
    v_dram = nc.dram_tensor("v_dram", [HEADS, GUARD + PLANE4], BF16)
    # idxC[q, h, p, m] int16 (p-major: partition (h,p) stride 16 -> 1 DMA)
    idxC = nc.dram_tensor("idxC", [16, HEADS, P, 16], I16)

    vwrite_insts = []
    vwrite_quarter = [[], [], [], []]
    cwrite_insts = []
    gather_insts = []
    idxload_insts = []

    with tile.TileContext(nc) as tc:
        import contextlib
        with contextlib.ExitStack() as ctx:
            persist = ctx.enter_context(tc.tile_pool(name="persist", bufs=1))
            ws = ctx.enter_context(tc.tile_pool(name="wstream", bufs=2))
            h1p = ctx.enter_context(tc.tile_pool(name="h1p", bufs=1))
            ctxp = ctx.enter_context(tc.tile_pool(name="ctxp", bufs=1))
            vsbp = ctx.enter_context(tc.tile_pool(name="vsbp", bufs=2))
            # g bufs must be >= the prep group size (4): with fewer, a prep's
            # WAR on its slot would wait on a reduce whose trigger comes later
            # in Pool program order -> deadlock.
            gp = ctx.enter_context(tc.tile_pool(name="gp", bufs=4))
            wtp = ctx.enter_context(tc.tile_pool(name="wtp", bufs=1))
            scr = ctx.enter_context(tc.tile_pool(name="scr", bufs=1))
            mps = ctx.enter_context(tc.tile_pool(name="mps", bufs=2, space="PSUM"))
            vps = ctx.enter_context(tc.tile_pool(name="vps", bufs=2, space="PSUM"))
            tps = ctx.enter_context(tc.tile_pool(name="tps", bufs=2, space="PSUM"))
            if repeat > 1:
                ctx.enter_context(tc.For_i(0, repeat, 1))

            # ---------- persistent loads ----------
            def load_tiles(dram, rows, cols, dt, tag, ncols=None):
                ncols = cols if ncols is None else ncols
                ts_ = []
                for k in range(rows // 128):
                    t = persist.tile([128, ncols], dt, tag=f"{tag}_{k}")
                    nc.sync.dma_start(t[:], dram[k * 128:(k + 1) * 128, :])
                    ts_.append(t)
                return ts_

            # Wv as ONE wide DMA (HWDGE fixed cost ~625ns per dma_start is a
            # serialized shared resource -- minimize dma_start count).
            def load_tiles1(dram, rows, cols, dt, tag):
                big = persist.tile([128, (rows // 128) * cols], dt, tag=tag,
                                   name=tag)
                kt = rows // 128
                nc.sync.dma_start(
                    bass.AP(big[:].tensor, big[:].offset,
                            [[kt * cols, 128], [cols, kt], [1, cols]]),
                    bass.AP(dram.ap().tensor, 0,
                            [[cols, 128], [128 * cols, kt], [1, cols]]))
                return [big[:, k * cols:(k + 1) * cols] for k in range(kt)]

            wv_hi = load_tiles1(Wv_hi, DIM, INNER, BF16, "wvh")
            wv_lo = load_tiles1(Wv_lo, DIM, INNER, BF16, "wvl") if V_PASSES == 3 else None

            # ctx supertiles loaded up-front so stage E matmuls can start
            # while the MLP stages run.
            def load_ctx(dram, c0, tag):
                big = ctxp.tile([128, KT * CTX_SUP], BF16, tag=tag, name=tag)
                nc.sync.dma_start(
                    bass.AP(big[:].tensor, big[:].offset,
                            [[KT * CTX_SUP, 128], [CTX_SUP, KT], [1, CTX_SUP]]),
                    bass.AP(dram.ap().tensor, c0,
                            [[CTX, 128], [128 * CTX, KT], [1, CTX_SUP]]))
                return [big[:, k * CTX_SUP:(k + 1) * CTX_SUP]
                        for k in range(KT)]

            if STAGE_CAP >= 'e' and not SKIP_E:
                ctx_hi_t = [load_ctx(ctxT_hi, sup * CTX_SUP, f"chbig_{sup}")
                            for sup in range(N_SUP)]
            def load_wide(dram, cols, dt, tag):
                big = persist.tile([128, KT * cols], dt, tag=tag, name=tag)
                nc.sync.dma_start(
                    bass.AP(big[:].tensor, big[:].offset,
                            [[KT * cols, 128], [cols, KT], [1, cols]]),
                    bass.AP(dram.ap().tensor, 0,
                            [[cols, 128], [128 * cols, KT], [1, cols]]))
                return [big[:, k * cols:(k + 1) * cols] for k in range(KT)]

            woff2 = load_wide(W_off2p, 256, F32R, "wo2big")
            watt2 = load_wide(W_att2, 128, BF16, "wa2big")

            def load_bias(dram, n_elem, tag):
                k = n_elem // 128
                t = persist.tile([128, k], F32, tag=tag)
                nc.sync.dma_start(t[:], _ap(dram, 0, [[1, 128], [128, k]]))
                return t

            bo1 = load_bias(b_off1, DIM, "bo1")
            bo2 = load_bias(b_off2p, 256, "bo2")
            ba1 = load_bias(b_att1, DIM, "ba1")
            ba2 = load_bias(b_att2, 128, "ba2")
            bo = load_bias(b_out, DIM, "bo")

            ident = persist.tile([128, 128], F32, tag="ident")
            make_identity(nc, ident[:])
            identb = persist.tile([128, 128], BF16, tag="identb")
            make_identity(nc, identb[:])

            # ---------- MLP helper: yT[m] = act(sum_k W[k,m]^T @ xT[k] + b[m]) ----------
            def mlp_layer(w_dram, x_tiles, bias_tile, mtiles, act, out_tag,
                          pool, dt, out_dt):
                # w_dram is [DIM, mtiles*128]; one coalesced DMA per m-tile
                wcols = mtiles * 128
                outs = []
                for m in range(mtiles):
                    ps = mps.tile([128, N], F32, tag="mlp_ps")
                    wt = ws.tile([128, 1024], dt, tag="wst",
                                 name=f"wst_{out_tag}_{m}")
                    src_ap = bass.AP(w_dram.ap().tensor, m * 128,
                                     [[wcols, 128], [128 * wcols, KT], [1, 128]])
                    nc.sync.dma_start(
                        bass.AP(wt[:].tensor, wt[:].offset,
                                [[1024, 128], [128, KT], [1, 128]]),
                        src_ap)
                    for k in range(KT):
                        nc.tensor.matmul(ps[:], wt[:, k * 128:(k + 1) * 128],
                                         x_tiles[k][:],
                                         start=(k == 0), stop=(k == KT - 1))
                    o = pool.tile([128, N], out_dt, tag=f"{out_tag}_{m}")
                    nc.scalar.activation(o[:], ps[:], act,
                                         bias=bias_tile[:, m:m + 1])
                    outs.append(o)
                return outs

            def mlp_layer2(w_tiles, x_tiles, bias_tile, mtiles, act, out_tag,
                           use_dve_bias=False):
                outs = []
                for m in range(mtiles):
                    ps = mps.tile([128, N], F32, tag="mlp_ps")
                    for k in range(KT):
                        nc.tensor.matmul(ps[:], w_tiles[k][:, m * 128:(m + 1) * 128],
                                         x_tiles[k][:],
                                         start=(k == 0), stop=(k == KT - 1))
                    o = scr.tile([128, N], F32, tag=f"{out_tag}_{m}")
                    if use_dve_bias:
                        nc.vector.tensor_scalar(o[:], ps[:], bias_tile[:, m:m + 1],
                                                None, op0=ALU.add)
                    else:
                        nc.scalar.activation(o[:], ps[:], act,
                                             bias=bias_tile[:, m:m + 1])
                    outs.append(o)
                return outs

            # ---------- stage A: offset MLP (fp32r) ----------
            def load_xT(dram, dt, name):
                big = h1p.tile([128, KT * N], dt, tag="xt_big", name=name)
                nc.sync.dma_start(
                    bass.AP(big[:].tensor, big[:].offset,
                            [[KT * N, 128], [N, KT], [1, N]]),
                    bass.AP(dram.ap().tensor, 0,
                            [[N, 128], [128 * N, KT], [1, N]]))
                return [big[:, k * N:(k + 1) * N] for k in range(KT)]

            xoff_t = load_xT(xoffT, F32R, "xoff_big")
            h1 = mlp_layer(W_off1, xoff_t, bo1, KT, AF.Gelu, "h1", h1p,
                           dt=F32R, out_dt=F32R)
            # off2: 2 m-tiles -> lxT (cy=0), lyT (cy=1), tanh applied
            loff = mlp_layer2(woff2, h1, bo2, 2, AF.Tanh, "loff")
            lxT, lyT = loff

            # ---------- stage D1 (moved early): positions -> gather indices.
            # Runs right after stage A so the idx tile is ready ~20us in and
            # the SWDGE descriptor pre-generation (stage F preps) can overlap
            # all of stage E. ----------
            def pos_chain_T(lt, tag):
                # g = clamp(tanh*31.5 + 31.5); xf = floor(g) exactly,
                # independent of the hw/sim cast rounding mode:
                # xi = cast(g); xf = float(xi); m = (g - xf < 0); xf -= m
                gp_ = scr.tile([128, N], F32, tag="tmpA", name=f"{tag}_g")
                nc.vector.tensor_scalar(gp_[:], lt[:], 31.5, 31.5,
                                        op0=ALU.mult, op1=ALU.add)
                nc.vector.tensor_scalar(gp_[:], gp_[:], 62.9999, 0.0,
                                        op0=ALU.min, op1=ALU.max)
                xi = scr.tile([128, N], I32, tag="tmpB", name=f"{tag}_i")
                nc.vector.tensor_copy(xi[:], gp_[:])
                xf = scr.tile([128, N], F32, tag=f"{tag}_f")
                nc.vector.tensor_copy(xf[:], xi[:])
                wr = scr.tile([128, N], F32, tag="tmpC", name=f"{tag}_wr")
                nc.vector.tensor_tensor(wr[:], gp_[:], xf[:], op=ALU.subtract)
                mneg = scr.tile([128, N], F32, tag="tmpD", name=f"{tag}_m")
                nc.vector.tensor_scalar(mneg[:], wr[:], 0.0, None, op0=ALU.is_lt)
                nc.vector.tensor_tensor(xf[:], xf[:], mneg[:], op=ALU.subtract)
                return xf

            xfT = pos_chain_T(lxT, "pxT")
            yfT = pos_chain_T(lyT, "pyT")
            idx0f = scr.tile([128, N], F32, tag="tmpA", name="idx0f")
            nc.vector.scalar_tensor_tensor(idx0f[:], yfT[:], 64.0, xfT[:],
                                           op0=ALU.mult, op1=ALU.add)
            ii = scr.tile([128, N], I16, tag="tmpB", name="idxi")
            nc.vector.tensor_copy(ii[:], idx0f[:])
            # free transpose: Sg[hp, q*16+m] = ii[hp, m*16+q]
            sg = scr.tile([128, N], I16, tag="sg_0")
            nc.vector.tensor_copy(
                sg[:], _sap(ii, 0, [[1, 16], [16, 16]]))
            # write to idxC[q, h, p, m]: partition (h,p) has constant
            # stride 16, so one DMA covers all 128 partitions x (q, m)
            dst = bass.AP(idxC.ap().tensor, 0,
                          [[16, 128], [2048, 16], [1, 16]])
            w = nc.sync.dma_start(dst, sg[:])
            cwrite_insts.append(w)

            # ---------- stage B: attention MLP (bf16) ----------
            xatt_t = load_xT(xattT, BF16, "xatt_big")
            g1 = mlp_layer(W_att1, xatt_t, ba1, KT, AF.Gelu, "h1", h1p,
                           dt=BF16, out_dt=BF16)
            attT = mlp_layer2(watt2, g1, ba2, 1, AF.Copy, "attT",
                              use_dve_bias=True)[0]

            # ---------- stage C: PE transposes to [n, hp] ----------
            def transpose_128x256(src, tag):
                halves = []
                for i in range(2):
                    pt = tps.tile([128, 128], F32, tag="trps")
                    nc.tensor.transpose(pt[:], src[:, i * 128:(i + 1) * 128],
                                        ident[:])
                    o = scr.tile([128, 128], F32, tag=f"{tag}_{i}")
                    nc.vector.tensor_copy(o[:], pt[:])
                    halves.append(o)
                return halves

            lx_n = transpose_128x256(lxT, "lxn")   # [n-tile][128, 128hp]
            ly_n = transpose_128x256(lyT, "lyn")
            att_n = transpose_128x256(attT, "attn")

            # ---------- stage D2: lerp weights + softmax in [n, hp] ----------
            # t[n, col = h*32 + cy*16 + p*2 + half]
            t_tiles = []
            for nh in range(2):
                def frac_chain(src_t, tag, otag):
                    g_ = scr.tile([128, 128], F32, tag="tmpA", name=f"{tag}_g")
                    nc.vector.tensor_scalar(g_[:], src_t[:], 31.5, 31.5,
                                            op0=ALU.mult, op1=ALU.add)
                    nc.vector.tensor_scalar(g_[:], g_[:], 62.9999, 0.0,
                                            op0=ALU.min, op1=ALU.max)
                    i_ = scr.tile([128, 128], I32, tag="tmpB", name=f"{tag}_i")
                    nc.vector.tensor_copy(i_[:], g_[:])
                    f_ = scr.tile([128, 128], F32, tag="tmpC", name=f"{tag}_f")
                    nc.vector.tensor_copy(f_[:], i_[:])
                    wr_ = scr.tile([128, 128], F32, tag="tmpD", name=f"{tag}_wr")
                    nc.vector.tensor_tensor(wr_[:], g_[:], f_[:], op=ALU.subtract)
                    mn_ = scr.tile([128, 128], F32, tag="tmpA", name=f"{tag}_m")
                    nc.vector.tensor_scalar(mn_[:], wr_[:], 0.0, None, op0=ALU.is_lt)
                    w_ = scr.tile([128, 128], F32, tag=otag, name=f"{tag}_w")
                    nc.vector.tensor_tensor(w_[:], wr_[:], mn_[:], op=ALU.add)
                    return w_
                wx = frac_chain(lx_n[nh], f"fx{nh}", "wx")
                wy = frac_chain(ly_n[nh], f"fy{nh}", "wy")

                # softmax over p (groups of 8 along free)
                an = att_n[nh]
                mx = scr.tile([128, 16], F32, tag="mx", name=f"mx_{nh}")
                nc.vector.tensor_reduce(
                    mx[:], _sap(an, 0, [[8, 16], [1, 8]]),
                    axis=AX.X, op=ALU.max)
                ex = scr.tile([128, 128], F32, tag="tmpB", name=f"ex_{nh}")
                nc.vector.tensor_tensor(
                    _sap(ex, 0, [[8, 16], [1, 8]]),
                    _sap(an, 0, [[8, 16], [1, 8]]),
                    _sap(mx, 0, [[1, 16], [0, 8]]),
                    op=ALU.subtract)
                nc.scalar.activation(ex[:], ex[:], AF.Exp)
                sm = scr.tile([128, 16], F32, tag="sm", name=f"sm_{nh}")
                nc.vector.tensor_reduce(
                    sm[:], _sap(ex, 0, [[8, 16], [1, 8]]),
                    axis=AX.X, op=ALU.add)
                rs = scr.tile([128, 16], F32, tag="rs", name=f"rs_{nh}")
                nc.vector.reciprocal(rs[:], sm[:])
                aw = scr.tile([128, 128], F32, tag="tmpC", name=f"aw_{nh}")
                nc.vector.tensor_tensor(
                    _sap(aw, 0, [[8, 16], [1, 8]]),
                    _sap(ex, 0, [[8, 16], [1, 8]]),
                    _sap(rs, 0, [[1, 16], [0, 8]]),
                    op=ALU.mult)

                # u0 = aw*(1-wx) = aw - aw*wx ; u1 = aw*wx
                u1 = scr.tile([128, 128], F32, tag="tmpD", name=f"u1_{nh}")
                nc.vector.tensor_tensor(u1[:], aw[:], wx[:], op=ALU.mult)
                u0 = scr.tile([128, 128], F32, tag="tmpA", name=f"u0_{nh}")
                nc.vector.tensor_tensor(u0[:], aw[:], u1[:], op=ALU.subtract)
                cw1 = wy
                cw0 = scr.tile([128, 128], F32, tag="tmpB", name=f"cw0_{nh}")
                nc.vector.tensor_scalar(cw0[:], wy[:], -1.0, 1.0,
                                        op0=ALU.mult, op1=ALU.add)

                # t col = h*32 + p*4 + cy*2 + half (p-major, matching the
                # idxC item order p*32 + cy*16 + m)
                tt = scr.tile([128, 512], BF16, tag=f"tt_{nh}")
                for cyv, cw in ((0, cw0), (1, cw1)):
                    for half, u in ((0, u0), (1, u1)):
                        nc.vector.tensor_tensor(
                            _sap(tt, cyv * 2 + half, [[32, 16], [4, 8]]),
                            _sap(u, 0, [[8, 16], [1, 8]]),
                            _sap(cw, 0, [[8, 16], [1, 8]]),
                            op=ALU.mult)
                t_tiles.append(tt)

            if STAGE_CAP >= 'e' and not SKIP_E:
                # ---------- stage E: v matmul + store (head-quarter outer so
                # the gather for heads 4g..4g+3 overlaps later quarters'
                # matmuls).  Each half-quarter (2048 positions) is staged in
                # ONE SBUF buffer and stored with 4 coalesced DMAs (one per
                # bilinear delta) -- dma_start count is the serialized HWDGE
                # bottleneck (~625ns each), so 512 stores -> 44. ----------
                HM = 16          # mm-tiles per half-quarter
                for qg in range(4):
                    for half in range(2):
                        vq = vsbp.tile([128, HM * 256], BF16, tag="vsb",
                                       name=f"vsb_{qg}_{half}")
                        for j in range(HM):
                            sup = half * 4 + j // M_PER_SUP
                            mm = j % M_PER_SUP
                            msl = slice(mm * 128, (mm + 1) * 128)
                            ps = vps.tile([128, 256], F32, tag="vps",
                                          name=f"vps_{qg}_{sup}_{mm}")
                            for k in range(KT):
                                nc.tensor.matmul(
                                    ps[:], ctx_hi_t[sup][k][:, msl],
                                    wv_hi[k][:, qg * 256:(qg + 1) * 256],
                                    start=(k == 0), stop=(k == KT - 1))
                            nc.scalar.copy(vq[:, j * 256:(j + 1) * 256], ps[:])
                        # quad plane: position p lands in chunk p-d at slot d
                        # for d in {0 (self), 1 (x+1), 64 (y+1), 65 (x+1,y+1)}.
                        # chunks below 0 land in the guard band (never read),
                        # so no edge special-casing.  3-dim AP limit => one
                        # DMA per (delta, head).
                        p0 = half * 2048
                        PL = GUARD + PLANE4
                        for dlt, soff in ((0, 0), (1, DH), (64, 2 * DH),
                                          (65, 3 * DH)):
                            for h in range(4):
                                dst = bass.AP(
                                    v_dram.ap().tensor,
                                    (qg * 4 + h) * PL + GUARD + soff
                                    + (p0 - dlt) * 256,
                                    [[256, 128], [128 * 256, HM], [1, DH]])
                                w = nc.sync.dma_start(
                                    dst, _sap(vq, h * DH,
                                              [[256, HM], [1, DH]]))
                                vwrite_quarter[qg].append(w)
                                vwrite_insts.append(w)

            if STAGE_CAP >= 'f':
                # ---------- stage F: per-head gather + reduce ----------
                # z reuses the xt_big slot (dead after stage B MLPs); bf16 so the
                # segmented reduce qualifies for the DVE 2x_1P perf mode
                z = h1p.tile([128, 2048], BF16, tag="xt_big", name="z")  # col = nh*1024 + h*64 + d
                # all-heads idx tile [128, 2048]: col = h*128 + p*16 + m,
                # partitions = 8 replicas x 16 q (one coalesced DMA)
                all_idx = persist.tile([128, 2048], I16, tag="all_idx")
                src = bass.AP(idxC.ap().tensor, 0,
                              [[0, 8], [2048, 16], [1, 2048]])
                ld = nc.sync.dma_start(
                    bass.AP(all_idx[:].tensor, all_idx[:].offset,
                            [[2048, 128], [1, 2048]]),
                    src)
                idxload_insts.append(ld)

                g_tiles = []
                for h in range(GHEADS):
                    g = gp.tile([128, 16, 256], BF16, tag="g", name=f"g_{h}")
                    vsrc = bass.AP(v_dram.ap().tensor,
                                   h * (GUARD + PLANE4) + GUARD,
                                   [[256, CTX - 65], [1, 256]])
                    gi = nc.gpsimd.dma_gather(
                        g[:], vsrc, all_idx[:, h * 128:(h + 1) * 128],
                        2048, 2048, 256, elem_step=256,
                        single_packet=SPKT)
                    gather_insts.append(gi)
                    g_tiles.append(g)

                for h in range(GHEADS if not GONLY else 0):
                    g = g_tiles[h]
                    for nh in range(2):
                        wt = wtp.tile([128, 2048], BF16, tag="wt")
                        # wt[d*32 + p*4 + slot] = G[n, slab 2p+nh, slot*64+d] * t
                        nc.vector.tensor_tensor(
                            _sap(wt, 0, [[32, 64], [4, 8], [1, 4]]),
                            _sap(g, nh * 256, [[1, 64], [512, 8], [64, 4]]),
                            _sap(t_tiles[nh], h * 32, [[0, 64], [4, 8], [1, 4]]),
                            op=ALU.mult)
                        with nc.allow_low_precision(
                                reason="bf16 z: 2x DVE reduce; rounded to bf16 "
                                       "for the out-proj matmul anyway"):
                            nc.vector.tensor_reduce(
                                _sap(z, nh * 1024 + h * 64, [[1, 64]]),
                                _sap(wt, 0, [[32, 64], [1, 32]]),
                                axis=AX.X, op=ALU.add)

            if STAGE_CAP >= 'g':
                # ---------- stage G: z -> zT, out projection (bf16) ----------
                # zT reuses the wo2big slot (dead after stage A MLPs)
                zbig = persist.tile([128, KT * N], BF16, tag="wo2big", name="zbig")
                zT = [zbig[:, k * N:(k + 1) * N] for k in range(KT)]
                for nh in range(2):
                    for c in range(8):
                        pt = tps.tile([128, 128], BF16, tag="trps")
                        nc.tensor.transpose(
                            pt[:], z[:, nh * 1024 + c * 128: nh * 1024 + (c + 1) * 128],
                            identb[:])
                        # ACT copy: keep DVE free for the reduce tail
                        nc.scalar.copy(zT[c][:, nh * 128:(nh + 1) * 128], pt[:])

                osb_big = scr.tile([128, KT * N], F32, tag="osb")
                for m in range(KT):
                    ps = mps.tile([128, N], F32, tag="mlp_ps")
                    wt = ws.tile([128, 1024], BF16, tag="wst", name=f"wst_out_{m}")
                    nc.sync.dma_start(
                        bass.AP(wt[:].tensor, wt[:].offset,
                                [[1024, 128], [128, KT], [1, 128]]),
                        bass.AP(W_out.ap().tensor, m * 128,
                                [[DIM, 128], [128 * DIM, KT], [1, 128]]))
                    for k in range(KT):
                        nc.tensor.matmul(ps[:], wt[:, k * 128:(k + 1) * 128],
                                         zT[k][:],
                                         start=(k == 0), stop=(k == KT - 1))
                    nc.vector.tensor_scalar(osb_big[:, m * N:(m + 1) * N],
                                            ps[:], bo[:, m:m + 1], None,
                                            op0=ALU.add)
                # outT as one coalesced store
                nc.sync.dma_start(
                    bass.AP(outT.ap().tensor, 0,
                            [[N, 128], [128 * N, KT], [1, N]]),
                    _sap(osb_big, 0, [[N, KT], [1, N]]))

            # ---------- explicit DRAM deps (gather after v/idx writes) ----------
            from concourse.tile_rust import add_dep_helper
            for hh, gi in enumerate(gather_insts):
                for w in vwrite_quarter[hh // 4]:
                    add_dep_helper(gi.ins, w.ins, reason="gather after v write")
            for ld in idxload_insts:
                for w in cwrite_insts:
                    add_dep_helper(ld.ins, w.ins, reason="idx load after idxC write")

    nc.compile()
    return nc


def _prep_inputs(inputs):
    x = np.ascontiguousarray(np.asarray(inputs["x"], dtype=np.float32))
    context = np.asarray(inputs["context"], dtype=np.float32)
    Wv = np.asarray(inputs["Wv"], dtype=np.float32)
    W_off2 = np.asarray(inputs["W_off2"], dtype=np.float32)
    b_off2 = np.asarray(inputs["b_off2"], dtype=np.float32)

    def split(a):
        import ml_dtypes
        hi = a.astype(ml_dtypes.bfloat16)
        lo = (a - hi.astype(np.float32)).astype(ml_dtypes.bfloat16)
        return hi, lo

    Wv_hi, Wv_lo = split(Wv)

    cy, h, p = np.meshgrid(np.arange(2), np.arange(HEADS), np.arange(P),
                           indexing="ij")
    oldcol = (h * 16 + p * 2 + cy).reshape(-1)
    W_off2p = np.ascontiguousarray(W_off2[:, oldcol])
    b_off2p = np.ascontiguousarray(b_off2[oldcol])

    import ml_dtypes
    bf = lambda a: np.ascontiguousarray(
        np.asarray(a, np.float32).astype(ml_dtypes.bfloat16))
    common = {
        "Wv_hi": Wv_hi,
        "W_off1": np.ascontiguousarray(np.asarray(inputs["W_off1"], np.float32)),
        "b_off1": np.asarray(inputs["b_off1"], np.float32),
        "W_off2p": W_off2p, "b_off2p": b_off2p,
        "W_att1": bf(inputs["W_att1"]),
        "b_att1": np.asarray(inputs["b_att1"], np.float32),
        "W_att2": bf(inputs["W_att2"]),
        "b_att2": np.asarray(inputs["b_att2"], np.float32),
        "W_out": bf(inputs["W_out"]),
        "b_out": np.asarray(inputs["b_out"], np.float32),
    }

    xflat = x.reshape(B * N, DIM)
    n_idx = np.arange(N)
    in_maps = []
    for b in range(B):
        perm = (n_idx // 32) * 256 + (n_idx % 32) * 8 + b
        xoffT = np.ascontiguousarray(xflat[perm].T)
        xattT = bf(x[b].T)
        ctxT = np.ascontiguousarray(context[b].T)
        c_hi, c_lo = split(ctxT)
        m = dict(common)
        m.update({"ctxT_hi": c_hi, "xoffT": xoffT, "xattT": xattT})
        if V_PASSES == 3:
            m.update({"ctxT_lo": c_lo, "Wv_lo": Wv_lo})
        in_maps.append(m)
    return in_maps


def kernel(**inputs):
    if "nc" not in _CACHE:
        _CACHE["nc"] = _build()
    nc = _CACHE["nc"]
    in_maps = _prep_inputs(inputs)
    res = run_bass_kernel_spmd(nc, in_maps, list(range(8)))
    _CACHE["last_results"] = res
    out = np.stack([res.results[i]["outT"].T for i in range(B)], axis=0)
    return np.ascontiguousarray(out.astype(np.float32))



# revision 25
# speedup vs baseline: 1.4921x; 1.4360x over previous
"""Trainium2 Bass kernel for nn_DeformableCrossAttention.

Sharding: data-parallel over batch B=8 across 8 NeuronCores (one sample per
core).  Inside each core:
  - offset MLP in fp32r (fp32 with 12-bit mantissa; full-rate 1 cyc/row on
    the PE vs 4 for fp32 -- position precision needs ~1e-4, fp32r gives it)
  - attention MLP + out-projection in bf16
  - v = context @ Wv single-pass bf16, quarter-split over heads so gathers
    for head group g start once that quarter of v is stored
  - bilinear sampling via per-head SWDGE dma_gather of 512B QUAD chunks
    (all 4 bilinear corners x 64 dh bf16) from 4x-shifted bf16 planes; the
    gather is descriptor-generation-bound (~7 ns/desc), so one descriptor
    per sample point is the key geometry
  - attention-weighted bilinear reduce on DVE (elementwise mult with
    factored corner weights + segmented add-reduce, bf16 out for 2x mode)
  - z transposed via PE (bf16), out-projection bf16, emitted transposed;
    host transposes back.

Self-contained: hardcodes all shapes from the problem spec.
"""
import sys
sys.path.insert(0, "/opt/trn_rl_repo")

import numpy as np
import concourse.bass as bass
import concourse.mybir as mybir
import concourse.tile as tile
from concourse import bacc
from concourse.bass_utils import run_bass_kernel_spmd
from concourse.masks import make_identity

F32 = mybir.dt.float32
F32R = mybir.dt.float32r
BF16 = mybir.dt.bfloat16
I16 = mybir.dt.int16
I32 = mybir.dt.int32
AF = mybir.ActivationFunctionType
ALU = mybir.AluOpType
AX = mybir.AxisListType

B, N, DIM = 8, 256, 1024
HEADS, DH, P = 16, 64, 8
HS = WS = 64
CTX = HS * WS            # 4096
INNER = HEADS * DH       # 1024
KT = DIM // 128          # 8 k-tiles
PLANE = CTX * DH         # per-head v plane elements (262144)

V_PASSES = int(__import__('os').environ.get('V_PASSES', '1'))  # 1: bf16, 3: bf16 hi/lo split (hh + hl + lh)
# timing bisection: run only pipeline stages up to this letter (d/e/f/g)
STAGE_CAP = __import__('os').environ.get('STAGE_CAP', 'g')
GONLY = __import__('os').environ.get('GONLY', '0') == '1'  # stage F: gather only
SPKT = __import__('os').environ.get('SPKT', '0') == '1'    # gather single_packet
SKIP_E = __import__('os').environ.get('SKIP_E', '0') == '1'  # timing: skip v matmul
GHEADS = int(__import__('os').environ.get('GHEADS', '16'))   # timing: gather count
GB512 = __import__('os').environ.get('GB512', '0') == '1'    # timing: 512B gather elems
CTX_SUP = 512            # ctx supertile rows
N_SUP = CTX // CTX_SUP   # 8 supertiles
M_PER_SUP = CTX_SUP // 128

_CACHE = {}


def _ap(t, offset, dims):
    return bass.AP(t.ap().tensor if hasattr(t, "ap") else t.tensor, offset, dims)


def _sap(tile_obj, extra, dims, part=None):
    """Sub-AP of an SBUF tile: keep its partition dim, custom free dims,
    extra offset in elements.  part=(off, cnt) slices the partition dim."""
    a = tile_obj[:] if not isinstance(tile_obj, bass.AP) else tile_obj
    if part is None:
        return bass.AP(a.tensor, a.offset + extra, [list(a.ap[0])] + dims)
    pstride = a.ap[0][0]
    return bass.AP(a.tensor, a.offset + extra + part[0] * pstride,
                   [[pstride, part[1]]] + dims)


def _build(repeat=1):
    nc = bacc.Bacc("TRN2", target_bir_lowering=False, debug=False)

    # ---------------- I/O ----------------
    ctxT_hi = nc.dram_tensor("ctxT_hi", [DIM, CTX], BF16, kind="ExternalInput")
    Wv_hi = nc.dram_tensor("Wv_hi", [DIM, INNER], BF16, kind="ExternalInput")
    if V_PASSES == 3:
        ctxT_lo = nc.dram_tensor("ctxT_lo", [DIM, CTX], BF16,
                                 kind="ExternalInput")
        Wv_lo = nc.dram_tensor("Wv_lo", [DIM, INNER], BF16,
                               kind="ExternalInput")
    else:
        ctxT_lo = Wv_lo = None
    # off path runs in fp32r (fp32 with 12-bit mantissa; full-rate on PE);
    # att + out-proj paths run in bf16.
    xoffT = nc.dram_tensor("xoffT", [DIM, N], F32R, kind="ExternalInput")
    xattT = nc.dram_tensor("xattT", [DIM, N], BF16, kind="ExternalInput")
    W_off1 = nc.dram_tensor("W_off1", [DIM, DIM], F32R, kind="ExternalInput")
    b_off1 = nc.dram_tensor("b_off1", [DIM], F32, kind="ExternalInput")
    W_off2p = nc.dram_tensor("W_off2p", [DIM, 256], F32R, kind="ExternalInput")
    b_off2p = nc.dram_tensor("b_off2p", [256], F32, kind="ExternalInput")
    W_att1 = nc.dram_tensor("W_att1", [DIM, DIM], BF16, kind="ExternalInput")
    b_att1 = nc.dram_tensor("b_att1", [DIM], F32, kind="ExternalInput")
    W_att2 = nc.dram_tensor("W_att2", [DIM, 128], BF16, kind="ExternalInput")
    b_att2 = nc.dram_tensor("b_att2", [128], F32, kind="ExternalInput")
    W_out = nc.dram_tensor("W_out", [INNER, DIM], BF16, kind="ExternalInput")
    b_out = nc.dram_tensor("b_out", [DIM], F32, kind="ExternalInput")

    outT = nc.dram_tensor("outT", [DIM, N], F32, kind="ExternalOutput")
    import os as _os
    DBG = _os.environ.get("KDBG", "0") == "1"
    if DBG:
        dbg_x = nc.dram_tensor("dbg_x", [128, 2048], F32, kind="ExternalOutput")
        dbg_h1 = nc.dram_tensor("dbg_h1", [128, N], F32, kind="ExternalOutput")
        dbg_lx = nc.dram_tensor("dbg_lx", [128, N], F32, kind="ExternalOutput")
        dbg_w = nc.dram_tensor("dbg_w", [128, 1024], F32, kind="ExternalOutput")

    # DRAM scratch.  v planes are bf16 QUAD chunks: chunk s = positions
    # {s, s+1, s+64, s+65} x 64 dh = 512B, so ONE SWDGE descriptor per
    # sample point fetches all 4 bilinear corners (the gather is
    # descriptor-generation-bound at ~7ns/desc, so halving descriptors
    # halves the gather wall time).
    PLANE4 = CTX * 256
    GUARD = 65 * 256         # negative-chunk guard band (written, never read)
    v_dram = nc.dram_tensor("v_dram", [HEADS, GUARD + PLANE4], BF16)
    # idxC[q, h, p, m] int16 (p-major: partition (h,p) stride 16 -> 1 DMA)
    idxC = nc.dram_tensor("idxC", [16, HEADS, P, 16], I16)

    vwrite_insts = []
    vwrite_quarter = [[], [], [], []]
    cwrite_insts = []
    gather_insts = []
    idxload_insts = []

    with tile.TileContext(nc) as tc:
        import contextlib
        with contextlib.ExitStack() as ctx:
            persist = ctx.enter_context(tc.tile_pool(name="persist", bufs=1))
            ws = ctx.enter_context(tc.tile_pool(name="wstream", bufs=2))
            h1p = ctx.enter_context(tc.tile_pool(name="h1p", bufs=1))
            ctxp = ctx.enter_context(tc.tile_pool(name="ctxp", bufs=1))
            vsbp = ctx.enter_context(tc.tile_pool(name="vsbp", bufs=2))
            gp = ctx.enter_context(tc.tile_pool(name="gp", bufs=4))
            wtp = ctx.enter_context(tc.tile_pool(name="wtp", bufs=1))
            scr = ctx.enter_context(tc.tile_pool(name="scr", bufs=1))
            mps = ctx.enter_context(tc.tile_pool(name="mps", bufs=2, space="PSUM"))
            vps = ctx.enter_context(tc.tile_pool(name="vps", bufs=2, space="PSUM"))
            tps = ctx.enter_context(tc.tile_pool(name="tps", bufs=2, space="PSUM"))
            if repeat > 1:
                ctx.enter_context(tc.For_i(0, repeat, 1))

            # ---------- persistent loads ----------
            def load_tiles(dram, rows, cols, dt, tag, ncols=None):
                ncols = cols if ncols is None else ncols
                ts_ = []
                for k in range(rows // 128):
                    t = persist.tile([128, ncols], dt, tag=f"{tag}_{k}")
                    nc.sync.dma_start(t[:], dram[k * 128:(k + 1) * 128, :])
                    ts_.append(t)
                return ts_

            # Wv as ONE wide DMA (HWDGE fixed cost ~625ns per dma_start is a
            # serialized shared resource -- minimize dma_start count).
            def load_tiles1(dram, rows, cols, dt, tag):
                big = persist.tile([128, (rows // 128) * cols], dt, tag=tag,
                                   name=tag)
                kt = rows // 128
                nc.sync.dma_start(
                    bass.AP(big[:].tensor, big[:].offset,
                            [[kt * cols, 128], [cols, kt], [1, cols]]),
                    bass.AP(dram.ap().tensor, 0,
                            [[cols, 128], [128 * cols, kt], [1, cols]]))
                return [big[:, k * cols:(k + 1) * cols] for k in range(kt)]

            wv_hi = load_tiles1(Wv_hi, DIM, INNER, BF16, "wvh")
            wv_lo = load_tiles1(Wv_lo, DIM, INNER, BF16, "wvl") if V_PASSES == 3 else None

            # ctx supertiles loaded up-front so stage E matmuls can start
            # while the MLP stages run.
            def load_ctx(dram, c0, tag):
                big = ctxp.tile([128, KT * CTX_SUP], BF16, tag=tag, name=tag)
                nc.sync.dma_start(
                    bass.AP(big[:].tensor, big[:].offset,
                            [[KT * CTX_SUP, 128], [CTX_SUP, KT], [1, CTX_SUP]]),
                    bass.AP(dram.ap().tensor, c0,
                            [[CTX, 128], [128 * CTX, KT], [1, CTX_SUP]]))
                return [big[:, k * CTX_SUP:(k + 1) * CTX_SUP]
                        for k in range(KT)]

            if STAGE_CAP >= 'e' and not SKIP_E:
                ctx_hi_t = [load_ctx(ctxT_hi, sup * CTX_SUP, f"chbig_{sup}")
                            for sup in range(N_SUP)]
            def load_wide(dram, cols, dt, tag):
                big = persist.tile([128, KT * cols], dt, tag=tag, name=tag)
                nc.sync.dma_start(
                    bass.AP(big[:].tensor, big[:].offset,
                            [[KT * cols, 128], [cols, KT], [1, cols]]),
                    bass.AP(dram.ap().tensor, 0,
                            [[cols, 128], [128 * cols, KT], [1, cols]]))
                return [big[:, k * cols:(k + 1) * cols] for k in range(KT)]

            woff2 = load_wide(W_off2p, 256, F32R, "wo2big")
            watt2 = load_wide(W_att2, 128, BF16, "wa2big")

            def load_bias(dram, n_elem, tag):
                k = n_elem // 128
                t = persist.tile([128, k], F32, tag=tag)
                nc.sync.dma_start(t[:], _ap(dram, 0, [[1, 128], [128, k]]))
                return t

            bo1 = load_bias(b_off1, DIM, "bo1")
            bo2 = load_bias(b_off2p, 256, "bo2")
            ba1 = load_bias(b_att1, DIM, "ba1")
            ba2 = load_bias(b_att2, 128, "ba2")
            bo = load_bias(b_out, DIM, "bo")

            ident = persist.tile([128, 128], F32, tag="ident")
            make_identity(nc, ident[:])
            identb = persist.tile([128, 128], BF16, tag="identb")
            make_identity(nc, identb[:])

            # ---------- MLP helper: yT[m] = act(sum_k W[k,m]^T @ xT[k] + b[m]) ----------
            def mlp_layer(w_dram, x_tiles, bias_tile, mtiles, act, out_tag,
                          pool, dt, out_dt):
                # w_dram is [DIM, mtiles*128]; one coalesced DMA per m-tile
                wcols = mtiles * 128
                outs = []
                for m in range(mtiles):
                    ps = mps.tile([128, N], F32, tag="mlp_ps")
                    wt = ws.tile([128, 1024], dt, tag="wst",
                                 name=f"wst_{out_tag}_{m}")
                    src_ap = bass.AP(w_dram.ap().tensor, m * 128,
                                     [[wcols, 128], [128 * wcols, KT], [1, 128]])
                    nc.sync.dma_start(
                        bass.AP(wt[:].tensor, wt[:].offset,
                                [[1024, 128], [128, KT], [1, 128]]),
                        src_ap)
                    for k in range(KT):
                        nc.tensor.matmul(ps[:], wt[:, k * 128:(k + 1) * 128],
                                         x_tiles[k][:],
                                         start=(k == 0), stop=(k == KT - 1))
                    o = pool.tile([128, N], out_dt, tag=f"{out_tag}_{m}")
                    nc.scalar.activation(o[:], ps[:], act,
                                         bias=bias_tile[:, m:m + 1])
                    outs.append(o)
                return outs

            def mlp_layer2(w_tiles, x_tiles, bias_tile, mtiles, act, out_tag,
                           use_dve_bias=False):
                outs = []
                for m in range(mtiles):
                    ps = mps.tile([128, N], F32, tag="mlp_ps")
                    for k in range(KT):
                        nc.tensor.matmul(ps[:], w_tiles[k][:, m * 128:(m + 1) * 128],
                                         x_tiles[k][:],
                                         start=(k == 0), stop=(k == KT - 1))
                    o = scr.tile([128, N], F32, tag=f"{out_tag}_{m}")
                    if use_dve_bias:
                        nc.vector.tensor_scalar(o[:], ps[:], bias_tile[:, m:m + 1],
                                                None, op0=ALU.add)
                    else:
                        nc.scalar.activation(o[:], ps[:], act,
                                             bias=bias_tile[:, m:m + 1])
                    outs.append(o)
                return outs

            # ---------- stage A: offset MLP (fp32r) ----------
            def load_xT(dram, dt, name):
                big = h1p.tile([128, KT * N], dt, tag="xt_big", name=name)
                nc.sync.dma_start(
                    bass.AP(big[:].tensor, big[:].offset,
                            [[KT * N, 128], [N, KT], [1, N]]),
                    bass.AP(dram.ap().tensor, 0,
                            [[N, 128], [128 * N, KT], [1, N]]))
                return [big[:, k * N:(k + 1) * N] for k in range(KT)]

            xoff_t = load_xT(xoffT, F32R, "xoff_big")
            h1 = mlp_layer(W_off1, xoff_t, bo1, KT, AF.Gelu, "h1", h1p,
                           dt=F32R, out_dt=F32R)
            # off2: 2 m-tiles -> lxT (cy=0), lyT (cy=1), tanh applied
            loff = mlp_layer2(woff2, h1, bo2, 2, AF.Tanh, "loff")
            lxT, lyT = loff

            # ---------- stage D1 (moved early): positions -> gather indices.
            # Runs right after stage A so the idx tile is ready ~20us in and
            # the SWDGE descriptor pre-generation (stage F preps) can overlap
            # all of stage E. ----------
            def pos_chain_T(lt, tag):
                # g = clamp(tanh*31.5 + 31.5); xf = floor(g) exactly,
                # independent of the hw/sim cast rounding mode:
                # xi = cast(g); xf = float(xi); m = (g - xf < 0); xf -= m
                gp_ = scr.tile([128, N], F32, tag="tmpA", name=f"{tag}_g")
                nc.vector.tensor_scalar(gp_[:], lt[:], 31.5, 31.5,
                                        op0=ALU.mult, op1=ALU.add)
                nc.vector.tensor_scalar(gp_[:], gp_[:], 62.9999, 0.0,
                                        op0=ALU.min, op1=ALU.max)
                xi = scr.tile([128, N], I32, tag="tmpB", name=f"{tag}_i")
                nc.vector.tensor_copy(xi[:], gp_[:])
                xf = scr.tile([128, N], F32, tag=f"{tag}_f")
                nc.vector.tensor_copy(xf[:], xi[:])
                wr = scr.tile([128, N], F32, tag="tmpC", name=f"{tag}_wr")
                nc.vector.tensor_tensor(wr[:], gp_[:], xf[:], op=ALU.subtract)
                mneg = scr.tile([128, N], F32, tag="tmpD", name=f"{tag}_m")
                nc.vector.tensor_scalar(mneg[:], wr[:], 0.0, None, op0=ALU.is_lt)
                nc.vector.tensor_tensor(xf[:], xf[:], mneg[:], op=ALU.subtract)
                return xf

            xfT = pos_chain_T(lxT, "pxT")
            yfT = pos_chain_T(lyT, "pyT")
            idx0f = scr.tile([128, N], F32, tag="tmpA", name="idx0f")
            nc.vector.scalar_tensor_tensor(idx0f[:], yfT[:], 64.0, xfT[:],
                                           op0=ALU.mult, op1=ALU.add)
            ii = scr.tile([128, N], I16, tag="tmpB", name="idxi")
            nc.vector.tensor_copy(ii[:], idx0f[:])
            # free transpose: Sg[hp, q*16+m] = ii[hp, m*16+q]
            sg = scr.tile([128, N], I16, tag="sg_0")
            nc.vector.tensor_copy(
                sg[:], _sap(ii, 0, [[1, 16], [16, 16]]))
            # write to idxC[q, h, p, m]: partition (h,p) has constant
            # stride 16, so one DMA covers all 128 partitions x (q, m).
            # Issued on the ACT HWDGE ring (nc.scalar) so the idx round-trip
            # doesn't queue behind the big SP-ring loads/stores: the gathers
            # wait on all_idx, and if this load sits behind the 128 v-store
            # DMAs in the FIFO the whole 277us gather chain serializes after
            # stage E (measured).
            dst = bass.AP(idxC.ap().tensor, 0,
                          [[16, 128], [2048, 16], [1, 16]])
            w = nc.scalar.dma_start(dst, sg[:])
            cwrite_insts.append(w)

            # all-heads idx tile [128, 2048]: col = h*128 + p*16 + m,
            # partitions = 8 replicas x 16 q (one coalesced DMA, loaded
            # here -- right after the write -- so it lands ~25us in)
            all_idx = persist.tile([128, 2048], I16, tag="all_idx")
            ld = nc.scalar.dma_start(
                bass.AP(all_idx[:].tensor, all_idx[:].offset,
                        [[2048, 128], [1, 2048]]),
                bass.AP(idxC.ap().tensor, 0,
                        [[0, 8], [2048, 16], [1, 2048]]))
            idxload_insts.append(ld)

            # ---------- stage B: attention MLP (bf16) ----------
            xatt_t = load_xT(xattT, BF16, "xatt_big")
            g1 = mlp_layer(W_att1, xatt_t, ba1, KT, AF.Gelu, "h1", h1p,
                           dt=BF16, out_dt=BF16)
            attT = mlp_layer2(watt2, g1, ba2, 1, AF.Copy, "attT",
                              use_dve_bias=True)[0]

            # ---------- stage C: PE transposes to [n, hp] ----------
            def transpose_128x256(src, tag):
                halves = []
                for i in range(2):
                    pt = tps.tile([128, 128], F32, tag="trps")
                    nc.tensor.transpose(pt[:], src[:, i * 128:(i + 1) * 128],
                                        ident[:])
                    o = scr.tile([128, 128], F32, tag=f"{tag}_{i}")
                    nc.vector.tensor_copy(o[:], pt[:])
                    halves.append(o)
                return halves

            lx_n = transpose_128x256(lxT, "lxn")   # [n-tile][128, 128hp]
            ly_n = transpose_128x256(lyT, "lyn")
            att_n = transpose_128x256(attT, "attn")

            # ---------- stage D2: lerp weights + softmax in [n, hp] ----------
            # t[n, col = h*32 + cy*16 + p*2 + half]
            t_tiles = []
            for nh in range(2):
                def frac_chain(src_t, tag, otag):
                    g_ = scr.tile([128, 128], F32, tag="tmpA", name=f"{tag}_g")
                    nc.vector.tensor_scalar(g_[:], src_t[:], 31.5, 31.5,
                                            op0=ALU.mult, op1=ALU.add)
                    nc.vector.tensor_scalar(g_[:], g_[:], 62.9999, 0.0,
                                            op0=ALU.min, op1=ALU.max)
                    i_ = scr.tile([128, 128], I32, tag="tmpB", name=f"{tag}_i")
                    nc.vector.tensor_copy(i_[:], g_[:])
                    f_ = scr.tile([128, 128], F32, tag="tmpC", name=f"{tag}_f")
                    nc.vector.tensor_copy(f_[:], i_[:])
                    wr_ = scr.tile([128, 128], F32, tag="tmpD", name=f"{tag}_wr")
                    nc.vector.tensor_tensor(wr_[:], g_[:], f_[:], op=ALU.subtract)
                    mn_ = scr.tile([128, 128], F32, tag="tmpA", name=f"{tag}_m")
                    nc.vector.tensor_scalar(mn_[:], wr_[:], 0.0, None, op0=ALU.is_lt)
                    w_ = scr.tile([128, 128], F32, tag=otag, name=f"{tag}_w")
                    nc.vector.tensor_tensor(w_[:], wr_[:], mn_[:], op=ALU.add)
                    return w_
                wx = frac_chain(lx_n[nh], f"fx{nh}", "wx")
                wy = frac_chain(ly_n[nh], f"fy{nh}", "wy")

                # softmax over p (groups of 8 along free)
                an = att_n[nh]
                mx = scr.tile([128, 16], F32, tag="mx", name=f"mx_{nh}")
                nc.vector.tensor_reduce(
                    mx[:], _sap(an, 0, [[8, 16], [1, 8]]),
                    axis=AX.X, op=ALU.max)
                ex = scr.tile([128, 128], F32, tag="tmpB", name=f"ex_{nh}")
                nc.vector.tensor_tensor(
                    _sap(ex, 0, [[8, 16], [1, 8]]),
                    _sap(an, 0, [[8, 16], [1, 8]]),
                    _sap(mx, 0, [[1, 16], [0, 8]]),
                    op=ALU.subtract)
                nc.scalar.activation(ex[:], ex[:], AF.Exp)
                sm = scr.tile([128, 16], F32, tag="sm", name=f"sm_{nh}")
                nc.vector.tensor_reduce(
                    sm[:], _sap(ex, 0, [[8, 16], [1, 8]]),
                    axis=AX.X, op=ALU.add)
                rs = scr.tile([128, 16], F32, tag="rs", name=f"rs_{nh}")
                nc.vector.reciprocal(rs[:], sm[:])
                aw = scr.tile([128, 128], F32, tag="tmpC", name=f"aw_{nh}")
                nc.vector.tensor_tensor(
                    _sap(aw, 0, [[8, 16], [1, 8]]),
                    _sap(ex, 0, [[8, 16], [1, 8]]),
                    _sap(rs, 0, [[1, 16], [0, 8]]),
                    op=ALU.mult)

                # u0 = aw*(1-wx) = aw - aw*wx ; u1 = aw*wx
                u1 = scr.tile([128, 128], F32, tag="tmpD", name=f"u1_{nh}")
                nc.vector.tensor_tensor(u1[:], aw[:], wx[:], op=ALU.mult)
                u0 = scr.tile([128, 128], F32, tag="tmpA", name=f"u0_{nh}")
                nc.vector.tensor_tensor(u0[:], aw[:], u1[:], op=ALU.subtract)
                cw1 = wy
                cw0 = scr.tile([128, 128], F32, tag="tmpB", name=f"cw0_{nh}")
                nc.vector.tensor_scalar(cw0[:], wy[:], -1.0, 1.0,
                                        op0=ALU.mult, op1=ALU.add)

                # t col = h*32 + p*4 + cy*2 + half (p-major, matching the
                # idxC item order p*32 + cy*16 + m)
                tt = scr.tile([128, 512], BF16, tag=f"tt_{nh}")
                for cyv, cw in ((0, cw0), (1, cw1)):
                    for half, u in ((0, u0), (1, u1)):
                        nc.vector.tensor_tensor(
                            _sap(tt, cyv * 2 + half, [[32, 16], [4, 8]]),
                            _sap(u, 0, [[8, 16], [1, 8]]),
                            _sap(cw, 0, [[8, 16], [1, 8]]),
                            op=ALU.mult)
                t_tiles.append(tt)

            if STAGE_CAP >= 'e' and not SKIP_E:
                # ---------- stage E: v matmul + store (head-quarter outer so
                # the gather for heads 4g..4g+3 overlaps later quarters'
                # matmuls).  Each half-quarter (2048 positions) is staged in
                # ONE SBUF buffer and stored with 4 coalesced DMAs (one per
                # bilinear delta) -- dma_start count is the serialized HWDGE
                # bottleneck (~625ns each), so 512 stores -> 44. ----------
                HM = 16          # mm-tiles per half-quarter
                for qg in range(4):
                    for half in range(2):
                        vq = vsbp.tile([128, HM * 256], BF16, tag="vsb",
                                       name=f"vsb_{qg}_{half}")
                        for j in range(HM):
                            sup = half * 4 + j // M_PER_SUP
                            mm = j % M_PER_SUP
                            msl = slice(mm * 128, (mm + 1) * 128)
                            ps = vps.tile([128, 256], F32, tag="vps",
                                          name=f"vps_{qg}_{sup}_{mm}")
                            for k in range(KT):
                                nc.tensor.matmul(
                                    ps[:], ctx_hi_t[sup][k][:, msl],
                                    wv_hi[k][:, qg * 256:(qg + 1) * 256],
                                    start=(k == 0), stop=(k == KT - 1))
                            nc.scalar.copy(vq[:, j * 256:(j + 1) * 256], ps[:])
                        # quad plane: position p lands in chunk p-d at slot d
                        # for d in {0 (self), 1 (x+1), 64 (y+1), 65 (x+1,y+1)}.
                        # chunks below 0 land in the guard band (never read),
                        # so no edge special-casing.  3-dim AP limit => one
                        # DMA per (delta, head).
                        p0 = half * 2048
                        PL = GUARD + PLANE4
                        for dlt, soff in ((0, 0), (1, DH), (64, 2 * DH),
                                          (65, 3 * DH)):
                            for h in range(4):
                                dst = bass.AP(
                                    v_dram.ap().tensor,
                                    (qg * 4 + h) * PL + GUARD + soff
                                    + (p0 - dlt) * 256,
                                    [[256, 128], [128 * 256, HM], [1, DH]])
                                w = nc.sync.dma_start(
                                    dst, _sap(vq, h * DH,
                                              [[256, HM], [1, DH]]))
                                vwrite_quarter[qg].append(w)
                                vwrite_insts.append(w)

            if STAGE_CAP >= 'f':
                # ---------- stage F: per-head gather + reduce ----------
                # z reuses the xt_big slot (dead after stage B MLPs); bf16 so the
                # segmented reduce qualifies for the DVE 2x_1P perf mode
                z = h1p.tile([128, 2048], BF16, tag="xt_big", name="z")  # col = nh*1024 + h*64 + d
                g_tiles = []
                for h in range(GHEADS):
                    g = gp.tile([128, 16, 256], BF16, tag="g", name=f"g_{h}")
                    vsrc = bass.AP(v_dram.ap().tensor,
                                   h * (GUARD + PLANE4) + GUARD,
                                   [[256, CTX - 65], [1, 256]])
                    gi = nc.gpsimd.dma_gather(
                        g[:], vsrc, all_idx[:, h * 128:(h + 1) * 128],
                        2048, 2048, 256, elem_step=256,
                        single_packet=SPKT)
                    gather_insts.append(gi)
                    g_tiles.append(g)

                for h in range(GHEADS if not GONLY else 0):
                    g = g_tiles[h]
                    for nh in range(2):
                        wt = wtp.tile([128, 2048], BF16, tag="wt")
                        # wt[d*32 + p*4 + slot] = G[n, slab 2p+nh, slot*64+d] * t
                        nc.vector.tensor_tensor(
                            _sap(wt, 0, [[32, 64], [4, 8], [1, 4]]),
                            _sap(g, nh * 256, [[1, 64], [512, 8], [64, 4]]),
                            _sap(t_tiles[nh], h * 32, [[0, 64], [4, 8], [1, 4]]),
                            op=ALU.mult)
                        with nc.allow_low_precision(
                                reason="bf16 z: 2x DVE reduce; rounded to bf16 "
                                       "for the out-proj matmul anyway"):
                            nc.vector.tensor_reduce(
                                _sap(z, nh * 1024 + h * 64, [[1, 64]]),
                                _sap(wt, 0, [[32, 64], [1, 32]]),
                                axis=AX.X, op=ALU.add)

            if STAGE_CAP >= 'g':
                # ---------- stage G: z -> zT, out projection (bf16) ----------
                # zT reuses the wo2big slot (dead after stage A MLPs)
                zbig = persist.tile([128, KT * N], BF16, tag="wo2big", name="zbig")
                zT = [zbig[:, k * N:(k + 1) * N] for k in range(KT)]
                for nh in range(2):
                    for c in range(8):
                        pt = tps.tile([128, 128], BF16, tag="trps")
                        nc.tensor.transpose(
                            pt[:], z[:, nh * 1024 + c * 128: nh * 1024 + (c + 1) * 128],
                            identb[:])
                        # ACT copy: keep DVE free for the reduce tail
                        nc.scalar.copy(zT[c][:, nh * 128:(nh + 1) * 128], pt[:])

                osb_big = scr.tile([128, KT * N], F32, tag="osb")
                for m in range(KT):
                    ps = mps.tile([128, N], F32, tag="mlp_ps")
                    wt = ws.tile([128, 1024], BF16, tag="wst", name=f"wst_out_{m}")
                    nc.sync.dma_start(
                        bass.AP(wt[:].tensor, wt[:].offset,
                                [[1024, 128], [128, KT], [1, 128]]),
                        bass.AP(W_out.ap().tensor, m * 128,
                                [[DIM, 128], [128 * DIM, KT], [1, 128]]))
                    for k in range(KT):
                        nc.tensor.matmul(ps[:], wt[:, k * 128:(k + 1) * 128],
                                         zT[k][:],
                                         start=(k == 0), stop=(k == KT - 1))
                    nc.vector.tensor_scalar(osb_big[:, m * N:(m + 1) * N],
                                            ps[:], bo[:, m:m + 1], None,
                                            op0=ALU.add)
                # outT as one coalesced store
                nc.sync.dma_start(
                    bass.AP(outT.ap().tensor, 0,
                            [[N, 128], [128 * N, KT], [1, N]]),
                    _sap(osb_big, 0, [[N, KT], [1, N]]))

            # ---------- explicit DRAM deps (gather after v/idx writes) ----------
            from concourse.tile_rust import add_dep_helper
            for hh, gi in enumerate(gather_insts):
                for w in vwrite_quarter[hh // 4]:
                    add_dep_helper(gi.ins, w.ins, reason="gather after v write")
            for ld in idxload_insts:
                for w in cwrite_insts:
                    add_dep_helper(ld.ins, w.ins, reason="idx load after idxC write")

    nc.compile()
    return nc


def _prep_inputs(inputs):
    x = np.ascontiguousarray(np.asarray(inputs["x"], dtype=np.float32))
    context = np.asarray(inputs["context"], dtype=np.float32)
    Wv = np.asarray(inputs["Wv"], dtype=np.float32)
    W_off2 = np.asarray(inputs["W_off2"], dtype=np.float32)
    b_off2 = np.asarray(inputs["b_off2"], dtype=np.float32)

    def split(a):
        import ml_dtypes
        hi = a.astype(ml_dtypes.bfloat16)
        lo = (a - hi.astype(np.float32)).astype(ml_dtypes.bfloat16)
        return hi, lo

    Wv_hi, Wv_lo = split(Wv)

    cy, h, p = np.meshgrid(np.arange(2), np.arange(HEADS), np.arange(P),
                           indexing="ij")
    oldcol = (h * 16 + p * 2 + cy).reshape(-1)
    W_off2p = np.ascontiguousarray(W_off2[:, oldcol])
    b_off2p = np.ascontiguousarray(b_off2[oldcol])

    import ml_dtypes
    bf = lambda a: np.ascontiguousarray(
        np.asarray(a, np.float32).astype(ml_dtypes.bfloat16))
    common = {
        "Wv_hi": Wv_hi,
        "W_off1": np.ascontiguousarray(np.asarray(inputs["W_off1"], np.float32)),
        "b_off1": np.asarray(inputs["b_off1"], np.float32),
        "W_off2p": W_off2p, "b_off2p": b_off2p,
        "W_att1": bf(inputs["W_att1"]),
        "b_att1": np.asarray(inputs["b_att1"], np.float32),
        "W_att2": bf(inputs["W_att2"]),
        "b_att2": np.asarray(inputs["b_att2"], np.float32),
        "W_out": bf(inputs["W_out"]),
        "b_out": np.asarray(inputs["b_out"], np.float32),
    }

    xflat = x.reshape(B * N, DIM)
    n_idx = np.arange(N)
    in_maps = []
    for b in range(B):
        perm = (n_idx // 32) * 256 + (n_idx % 32) * 8 + b
        xoffT = np.ascontiguousarray(xflat[perm].T)
        xattT = bf(x[b].T)
        ctxT = np.ascontiguousarray(context[b].T)
        c_hi, c_lo = split(ctxT)
        m = dict(common)
        m.update({"ctxT_hi": c_hi, "xoffT": xoffT, "xattT": xattT})
        if V_PASSES == 3:
            m.update({"ctxT_lo": c_lo, "Wv_lo": Wv_lo})
        in_maps.append(m)
    return in_maps


def kernel(**inputs):
    if "nc" not in _CACHE:
        _CACHE["nc"] = _build()
    nc = _CACHE["nc"]
    in_maps = _prep_inputs(inputs)
    res = run_bass_kernel_spmd(nc, in_maps, list(range(8)))
    _CACHE["last_results"] = res
    out = np.stack([res.results[i]["outT"].T for i in range(B)], axis=0)
    return np.ascontiguousarray(out.astype(np.float32))

